# revision 2
# baseline (speedup 1.0000x reference)
"""TRN2 Bass kernel for nn_CrossModalAttention_75316546503126.

Mathematical collapse exploited here (verified against the jax reference):
K/V rows of the attention are identical across the sequence axis because the
acoustic features are broadcast before the K/V projections.  Hence every
attention row sees a constant score vector, softmax is exactly uniform
(S = 2048 is a power of two, so 1/S is exact in fp32), and

    out[b, s, :] = text[b, s, :] @ Wt + bias_b
    bias_b       = bt + bv + (ac_b @ Wa + ba) @ Wv
                 = bias0 + ac_b @ (Wa @ Wv)          (weight-only terms fused
                                                      on the host into bias0
                                                      and Wav = Wa @ Wv)

i.e. one [S, D] x [D, D] matmul per batch plus a per-batch bias row.  All
data-dependent compute (text @ Wt, ac @ Wav) runs on device.

Sharding: data-parallel over batch B=8 across the 8 NeuronCores.

v2 implementation (Wt-stationary, transposed output):
  - Computes out^T = Wt^T @ X^T: each 128x128 Wt block is the PE stationary
    operand and streams 4x512 X^T columns per load, so the whole job is
    144 N=512 matmuls + ~150 LDWEIGHTS instead of the v1 layout's 229+234.
    Fewer instructions also shrink the NEFF's semaphore setup/teardown
    head+tail, which profiled at ~10us combined in v1.
  - out^T tiles are [128(d), 2048(s)] f32 in PSUM (4 banks); two live
    d-blocks fill all 8 banks.  Eviction adds the per-batch bias with a
    per-partition tensor_scalar_add (bias varies along d = partitions in the
    transposed layout) and downcasts to bf16; the host transposes back.
  - The bias is a K=16 matvec on device: biasT[d,1] = Wav^T ac^T per
    128-d block (6 tiny matmuls) + host-folded bias0^T, computed during the
    DMA-dead startup window right after the PE clock warm-up.
  - DMA order feeds the d0/d1 streaming phase: d01 weight columns first,
    then X^T k-slabs in halves (matmuls chase the DMA at half-slab
    granularity), then the d2..d5 weight columns which arrive before the
    streaming phase ends.  Loads ride the sync queue; tiny tensors and
    output stores ride the scalar queue; the last store splits across both.
  - WARMUP_MM dummy matmuls (one long PSUM accumulation group) bridge the
    ~8.8us DGE-dead window so the PE clock (HAM 0.65->1.2->2.4 GHz) is
    ramped when the first real matmul issues.
"""
import sys

if "/opt/trn_rl_repo" not in sys.path:
    sys.path.insert(0, "/opt/trn_rl_repo")

from contextlib import ExitStack

import numpy as np
import ml_dtypes

import concourse.bacc as bacc
import concourse.bass as bass
import concourse.mybir as mybir
import concourse.tile as tile
from concourse.bass_utils import run_bass_kernel_spmd

F32 = mybir.dt.float32
BF16 = mybir.dt.bfloat16

B, S, D = 8, 2048, 768
KB = D // 128           # 6 contraction blocks
DB = D // 128           # 6 output d-blocks
CS = 512                # psum-bank chunk of the s axis
NCH = S // CS           # 4 chunks
HS = S // 2             # half-slab columns
N_CORES = 8

WARMUP_MM = 4           # dummy [128,512] matmuls that ramp the PE clock

MODE = "bf16"


def build_program(mode=MODE):
    nc = bacc.Bacc()

    # xt[p, k*S + s] = X[s, k*128+p]   (k-major X^T slabs)
    xt = nc.declare_dram_parameter("xt", [128, KB * S], BF16, isOutput=False)
    # wt packed: cols [0,1536) = d01 columns k-major ([k*256+d], d<256),
    # cols [1536,4608) = d2..d5 columns k-major ([1536 + k*512 + (d-256)])
    wt = nc.declare_dram_parameter("wt", [128, KB * D], BF16, isOutput=False)
    ac = nc.declare_dram_parameter("ac", [1, 16], F32, isOutput=False)
    wav = nc.declare_dram_parameter("wav", [16, D], F32, isOutput=False)
    b0t = nc.declare_dram_parameter("b0t", [128, DB], F32, isOutput=False)
    outT = nc.declare_dram_parameter("outT", [D, S], BF16, isOutput=True)

    with tile.TileContext(nc) as tc, ExitStack() as ctx:
        const = ctx.enter_context(tc.tile_pool(name="const", bufs=1))
        wpool = ctx.enter_context(tc.tile_pool(name="wpool", bufs=1))
        xpool = ctx.enter_context(tc.tile_pool(name="xpool", bufs=1))
        opool = ctx.enter_context(tc.tile_pool(name="opool", bufs=3))
        # PSUM: 2 x [128, 2048] f32 = 8 banks; warm-up + bias share buf A
        pso = ctx.enter_context(tc.tile_pool(name="pso", bufs=2, space="PSUM"))

        # ---------------- PE warm-up fodder (no DMA dependencies) --------
        warm_w = const.tile([128, 128], BF16)
        nc.gpsimd.memset(warm_w[:], 1.0)
        warm_x = const.tile([128, 512], BF16)
        nc.gpsimd.memset(warm_x[:], 1.0)

        # ---------------- DMA schedule ----------------
        # scalar queue: tiny bias tensors first, output stores later
        acT = const.tile([16, 1], F32)
        nc.scalar.dma_start(acT[:], ac.rearrange("o k -> k o"))
        wav_sb = const.tile([16, D], F32)
        nc.scalar.dma_start(wav_sb[:], wav[:])
        b0t_sb = const.tile([128, DB], F32)
        nc.scalar.dma_start(b0t_sb[:], b0t[:])

        # sync queue: d01 weights (k0,k1 first), X^T half-slabs, d2..d5 wts
        wtA1 = wpool.tile([128, 512], BF16, tag="wtA1", name="wtA1")
        nc.sync.dma_start(wtA1[:], wt[:, 0:512])
        xh = [[None, None] for _ in range(KB)]

        def load_half(k, h):
            t = xpool.tile([128, HS], BF16, tag=f"x{k}{h}", name=f"x{k}{h}")
            nc.sync.dma_start(t[:], xt[:, k * S + h * HS:k * S + (h + 1) * HS])
            xh[k][h] = t

        load_half(0, 0)
        load_half(0, 1)
        wtA2 = wpool.tile([128, 1024], BF16, tag="wtA2", name="wtA2")
        nc.sync.dma_start(wtA2[:], wt[:, 512:1536])
        for k in range(1, KB):
            load_half(k, 0)
            load_half(k, 1)
        wtB = wpool.tile([128, 3072], BF16, tag="wtB", name="wtB")
        nc.sync.dma_start(wtB[:], wt[:, 1536:4608])

        def wt_block(k, db):
            if db < 2:
                if k < 2:
                    return wtA1[:, k * 256 + db * 128:k * 256 + (db + 1) * 128]
                return wtA2[:, (k - 2) * 256 + db * 128:
                            (k - 2) * 256 + (db + 1) * 128]
            off = k * 512 + (db - 2) * 128
            return wtB[:, off:off + 128]

        def x_chunk(k, c):
            return xh[k][c // 2][:, (c % 2) * CS:(c % 2) * CS + CS]

        # ---------------- PE warm-up + bias (DMA-dead window) ------------
        warm_ps = pso.tile([128, S], F32, tag="po", name="warm_ps")
        for i in range(WARMUP_MM):
            nc.tensor.matmul(warm_ps[:, 0:512], warm_w[:], warm_x[:],
                             start=(i == 0), stop=(i == WARMUP_MM - 1),
                             skip_group_check=True)

        # biasT[d, 1] per d-block: Wav^T @ ac^T (K=16), into warm bank 1
        for db in range(DB):
            nc.tensor.matmul(
                warm_ps[:, 512 + db:513 + db],
                wav_sb[:, db * 128:(db + 1) * 128],
                acT[:, :],
                start=True, stop=True,
            )
        biasT = const.tile([128, DB], F32)
        nc.vector.tensor_add(biasT[:], warm_ps[:, 512:512 + DB], b0t_sb[:])

        # ---------------- main d-block emitters ----------------
        def emit_dblock(db):
            ps = pso.tile([128, S], F32, tag="po", name=f"ps_d{db}")
            for k in range(KB):
                w = wt_block(k, db)
                for c in range(NCH):
                    nc.tensor.matmul(ps[:, c * CS:(c + 1) * CS], w,
                                     x_chunk(k, c),
                                     start=(k == 0), stop=(k == KB - 1))
            return ps

        def evict_store(db, ps):
            ot = opool.tile([128, S], BF16, tag="o")
            nc.vector.tensor_scalar_add(ot[:], ps[:, :], biasT[:, db:db + 1])
            nc.scalar.dma_start(outT[db * 128:(db + 1) * 128, :], ot[:])

        # streaming phase: d0 and d1 interleaved per k-slab so the PE
        # chases the X^T DMA at half-slab granularity
        ps0 = pso.tile([128, S], F32, tag="po", name="ps_d0")
        ps1 = pso.tile([128, S], F32, tag="po", name="ps_d1")
        for k in range(KB):
            for db, ps in ((0, ps0), (1, ps1)):
                w = wt_block(k, db)
                for c in range(NCH):
                    nc.tensor.matmul(ps[:, c * CS:(c + 1) * CS], w,
                                     x_chunk(k, c),
                                     start=(k == 0), stop=(k == KB - 1))
        evict_store(0, ps0)
        evict_store(1, ps1)

        # steady phase: d2..d4 from resident SBUF
        for db in range(2, DB - 1):
            evict_store(db, emit_dblock(db))

        # last d-block: per-chunk eviction + split store on both queues to
        # shorten the tail drain
        db = DB - 1
        ps = emit_dblock(db)
        ot = opool.tile([128, S], BF16, tag="o")
        for c in range(NCH):
            nc.vector.tensor_scalar_add(ot[:, c * CS:(c + 1) * CS],
                                        ps[:, c * CS:(c + 1) * CS],
                                        biasT[:, db:db + 1])
            if c == 1:
                nc.sync.dma_start(outT[db * 128:(db + 1) * 128, 0:S // 2],
                                  ot[:, 0:S // 2])
        nc.scalar.dma_start(outT[db * 128:(db + 1) * 128, S // 2:S],
                            ot[:, S // 2:S])

    nc.compile()
    return nc


_PROGRAM_CACHE = {}


def _get_program(mode=None):
    if mode is None:
        mode = MODE
    if mode not in _PROGRAM_CACHE:
        _PROGRAM_CACHE[mode] = build_program(mode)
    return _PROGRAM_CACHE[mode]


def make_in_maps(text_features, acoustic_features, Wt, bt, Wa, ba, Wv, bv):
    """Host-side sharding + layout prep: per-batch X^T k-slabs, the packed
    Wt blocks, and the host-fused bias terms (Wav = Wa @ Wv,
    bias0 = bt + bv + ba @ Wv), in the exact SBUF tile layouts."""
    bf16 = ml_dtypes.bfloat16
    text_features = np.asarray(text_features, dtype=np.float32)
    # xt[b, p, k*S + s] = X[b, s, k*128+p]
    xt_all = (text_features
              .reshape(B, S, KB, 128)
              .transpose(0, 3, 2, 1)
              .astype(bf16)
              .reshape(B, 128, KB * S))

    Wt3 = np.asarray(Wt, dtype=np.float32).reshape(KB, 128, D)
    wtA = Wt3[:, :, 0:256].transpose(1, 0, 2).reshape(128, KB * 256)
    wtB = Wt3[:, :, 256:768].transpose(1, 0, 2).reshape(128, KB * 512)
    wt_packed = np.concatenate([wtA, wtB], axis=1).astype(bf16)

    Wa = np.asarray(Wa, dtype=np.float32)
    Wv = np.asarray(Wv, dtype=np.float32)
    wav = np.ascontiguousarray(Wa @ Wv)
    bias0 = (np.asarray(bt, dtype=np.float32)
             + np.asarray(bv, dtype=np.float32)
             + np.asarray(ba, dtype=np.float32) @ Wv)
    b0t = np.ascontiguousarray(bias0.reshape(DB, 128).T)

    shared = {
        "wt": np.ascontiguousarray(wt_packed),
        "wav": wav,
        "b0t": b0t,
    }
    acoustic_features = np.ascontiguousarray(
        np.asarray(acoustic_features, dtype=np.float32))
    in_maps = []
    for b in range(N_CORES):
        m = dict(shared)
        m["xt"] = np.ascontiguousarray(xt_all[b])
        m["ac"] = acoustic_features[b:b + 1]
        in_maps.append(m)
    return in_maps


def kernel(text_features, acoustic_features, Wt, bt, Wa, ba, Wq, bq, Wk, bk,
           Wv, bv, **_unused):
    nc = _get_program()
    in_maps = make_in_maps(text_features, acoustic_features, Wt, bt, Wa, ba,
                           Wv, bv)
    res = run_bass_kernel_spmd(nc, in_maps, list(range(N_CORES))).results
    out = np.empty((B, S, D), dtype=np.float32)
    for b in range(N_CORES):
        out[b] = np.asarray(res[b]["outT"], dtype=np.float32).T
    return out


# revision 3
# speedup vs baseline: 1.1618x; 1.1618x over previous
"""TRN2 Bass kernel for nn_CrossModalAttention_75316546503126.

Mathematical collapse exploited here (verified against the jax reference):
K/V rows of the attention are identical across the sequence axis because the
acoustic features are broadcast before the K/V projections.  Hence every
attention row sees a constant score vector, softmax is exactly uniform
(S = 2048 is a power of two, so 1/S is exact in fp32), and

    out[b, s, :] = text[b, s, :] @ Wt + bias_b
    bias_b       = bias0 + ac_b @ Wav
    Wav          = Wa @ Wv,  bias0 = bt + bv + ba @ Wv   (weight-only fusions
                                                          done on the host)

i.e. one [S, D] x [D, D] matmul per batch plus a per-batch bias row.  All
data-dependent compute (text @ Wt, ac @ Wav) runs on device.

Sharding: data-parallel over batch B=8 across the 8 NeuronCores.

v3 implementation (Wt-stationary, transposed output, phase-split):
  - Computes out^T = Wt^T @ X^T: each 128x128 Wt block is the PE stationary
    operand and streams 512-column X^T chunks, so the whole job is 144 N=512
    matmuls at a measured 215 ns cadence (full 2.4 GHz, LDWEIGHTS fully
    pipelined) instead of v1's 229 matmuls + 234 weight reloads.
  - Phase split keeps the PE comfortably BEHIND the DMA stream (v2 paced
    them 1:1 and every micro-stall reset the HAM clock ramp to 1.2 GHz):
      P1: d-blocks 0..3 x s-chunks 0,1  -- needs only the first-half slabs,
          4 x [128,1024] PSUM tiles (8 banks), PE consumes 1.92us/slab vs
          ~0.75us/half-slab DMA arrival.
      P2: d-blocks 0..3 x s-chunks 2,3  -- second-half slabs, all resident.
      P3: d-blocks 4,5 full             -- everything resident.
    Weight k-slices interleave with the half-slabs so the first matmul only
    waits for wt(k0) + X^T(k0,h0) (~0.6 MB).
  - The per-batch bias is pure DVE work: 6 scalar_tensor_tensor ops with
    accum_out reduce Wav^T * ac (K=16) per d-block, plus the host-folded
    bias0^T; eviction adds it via per-partition tensor_scalar_add (bias
    varies along d = partitions in the transposed layout) and downcasts to
    bf16.  The host transposes the output back.
  - WARMUP_MM dummy matmuls (one long PSUM accumulation group) bridge the
    ~8.7us DGE-dead window so the PE clock is ramped when real work starts.
"""
import sys

if "/opt/trn_rl_repo" not in sys.path:
    sys.path.insert(0, "/opt/trn_rl_repo")

from contextlib import ExitStack

import numpy as np
import ml_dtypes

import concourse.bacc as bacc
import concourse.bass as bass
import concourse.mybir as mybir
import concourse.tile as tile
from concourse.bass_utils import run_bass_kernel_spmd

F32 = mybir.dt.float32
BF16 = mybir.dt.bfloat16
MULT = mybir.AluOpType.mult

B, S, D = 8, 2048, 768
KB = D // 128           # 6 contraction blocks
DB = D // 128           # 6 output d-blocks
CS = 512                # psum-bank chunk of the s axis
HS = S // 2             # half-slab columns
N_CORES = 8

WARMUP_MM = 7           # dummy [128,512] matmuls that ramp the PE clock

MODE = "bf16"


def build_program(mode=MODE):
    nc = bacc.Bacc()

    # xt[p, k*S + s] = X[s, k*128+p]   (k-major X^T slabs)
    xt = nc.declare_dram_parameter("xt", [128, KB * S], BF16, isOutput=False)
    # wt[p, k*D + d] = Wt[k*128+p, d]  (k-major weight slices)
    wt = nc.declare_dram_parameter("wt", [128, KB * D], BF16, isOutput=False)
    # wavT2[p, db*16+q] = Wav[q, db*128+p];  acB[p, q] = ac[q]
    wavT2 = nc.declare_dram_parameter("wavT2", [128, DB * 16], F32,
                                      isOutput=False)
    acB = nc.declare_dram_parameter("acB", [128, 16], F32, isOutput=False)
    b0t = nc.declare_dram_parameter("b0t", [128, DB], F32, isOutput=False)
    outT = nc.declare_dram_parameter("outT", [D, S], BF16, isOutput=True)

    with tile.TileContext(nc) as tc, ExitStack() as ctx:
        const = ctx.enter_context(tc.tile_pool(name="const", bufs=1))
        wpool = ctx.enter_context(tc.tile_pool(name="wpool", bufs=1))
        xpool = ctx.enter_context(tc.tile_pool(name="xpool", bufs=1))
        opool = ctx.enter_context(tc.tile_pool(name="opool", bufs=3))
        # PSUM: 4 x [128, 1024] f32 (2 banks each) = 8 banks
        pso = ctx.enter_context(tc.tile_pool(name="pso", bufs=4, space="PSUM"))

        # ---------------- PE warm-up fodder (no DMA dependencies) --------
        warm_w = const.tile([128, 128], BF16)
        nc.gpsimd.memset(warm_w[:], 1.0)
        warm_x = const.tile([128, 512], BF16)
        nc.gpsimd.memset(warm_x[:], 1.0)

        # ---------------- DMA schedule ----------------
        # scalar queue: tiny bias tensors first, output stores later
        acB_sb = const.tile([128, 16], F32)
        nc.scalar.dma_start(acB_sb[:], acB[:])
        wavT2_sb = const.tile([128, DB * 16], F32)
        nc.scalar.dma_start(wavT2_sb[:], wavT2[:])
        b0t_sb = const.tile([128, DB], F32)
        nc.scalar.dma_start(b0t_sb[:], b0t[:])

        # sync queue: (wt k-slice, X^T k-slab first half) pairs, then the
        # second halves -- P1 only touches the first halves
        wtk = []
        xh = [[None, None] for _ in range(KB)]
        for k in range(KB):
            t = wpool.tile([128, D], BF16, tag=f"wt{k}", name=f"wt{k}")
            nc.sync.dma_start(t[:], wt[:, k * D:(k + 1) * D])
            wtk.append(t)
            xtile = xpool.tile([128, HS], BF16, tag=f"x{k}0", name=f"x{k}0")
            nc.sync.dma_start(xtile[:], xt[:, k * S:k * S + HS])
            xh[k][0] = xtile
        for k in range(KB):
            xtile = xpool.tile([128, HS], BF16, tag=f"x{k}1", name=f"x{k}1")
            nc.sync.dma_start(xtile[:], xt[:, k * S + HS:(k + 1) * S])
            xh[k][1] = xtile

        def wt_block(k, db):
            return wtk[k][:, db * 128:(db + 1) * 128]

        # ---------------- PE warm-up (DMA-dead window) -------------------
        warm_ps = pso.tile([128, 2 * CS], F32, tag="po", name="warm_ps")
        for i in range(WARMUP_MM):
            nc.tensor.matmul(warm_ps[:, 0:CS], warm_w[:], warm_x[:],
                             start=(i == 0), stop=(i == WARMUP_MM - 1),
                             skip_group_check=True)

        # ---------------- bias on DVE only: biasT = b0t + Wav^T ac^T -----
        junk = const.tile([128, 16], F32)
        braw = const.tile([128, DB], F32)
        for db in range(DB):
            nc.vector.scalar_tensor_tensor(
                junk[:], wavT2_sb[:, db * 16:(db + 1) * 16], 1.0, acB_sb[:],
                MULT, MULT, accum_out=braw[:, db:db + 1])
        biasT = const.tile([128, DB], F32)
        nc.vector.tensor_add(biasT[:], braw[:], b0t_sb[:])

        # ---------------- main emitters ----------------
        def evict_store(db, ps, half, ot=None, store=True, eng=None):
            # half: 0 -> s columns [0,1024), 1 -> [1024,2048)
            if ot is None:
                ot = opool.tile([128, S], BF16, tag="o")
            dst = ot[:, half * HS:(half + 1) * HS]
            nc.vector.tensor_scalar_add(dst, ps[:, :], biasT[:, db:db + 1])
            if store:
                (eng or nc.scalar).dma_start(
                    outT[db * 128:(db + 1) * 128, half * HS:(half + 1) * HS],
                    dst)
            return ot

        # P1: d0..d3 on s-chunks 0,1 -- slab-paced, interleaved per k
        psA = [pso.tile([128, 2 * CS], F32, tag="po", name=f"p1d{d}")
               for d in range(4)]
        for k in range(KB):
            for d in range(4):
                w = wt_block(k, d)
                x0 = xh[k][0]
                nc.tensor.matmul(psA[d][:, 0:CS], w, x0[:, 0:CS],
                                 start=(k == 0), stop=(k == KB - 1))
                nc.tensor.matmul(psA[d][:, CS:2 * CS], w, x0[:, CS:2 * CS],
                                 start=(k == 0), stop=(k == KB - 1))
        p1_ot = []
        for d in range(4):
            p1_ot.append(evict_store(d, psA[d], 0))

        # P2: d0..d3 on s-chunks 2,3 -- solo per d-block (staggered evicts)
        for d in range(4):
            ps = pso.tile([128, 2 * CS], F32, tag="po", name=f"p2d{d}")
            for k in range(KB):
                w = wt_block(k, d)
                x1 = xh[k][1]
                nc.tensor.matmul(ps[:, 0:CS], w, x1[:, 0:CS],
                                 start=(k == 0), stop=(k == KB - 1))
                nc.tensor.matmul(ps[:, CS:2 * CS], w, x1[:, CS:2 * CS],
                                 start=(k == 0), stop=(k == KB - 1))
            evict_store(d, ps, 1)

        # P3: d4, d5 full
        for db in (4, 5):
            ps01 = pso.tile([128, 2 * CS], F32, tag="po", name=f"p3d{db}a")
            ps23 = pso.tile([128, 2 * CS], F32, tag="po", name=f"p3d{db}b")
            for k in range(KB):
                w = wt_block(k, db)
                nc.tensor.matmul(ps01[:, 0:CS], w, xh[k][0][:, 0:CS],
                                 start=(k == 0), stop=(k == KB - 1))
                nc.tensor.matmul(ps01[:, CS:2 * CS], w, xh[k][0][:, CS:2 * CS],
                                 start=(k == 0), stop=(k == KB - 1))
                nc.tensor.matmul(ps23[:, 0:CS], w, xh[k][1][:, 0:CS],
                                 start=(k == 0), stop=(k == KB - 1))
                nc.tensor.matmul(ps23[:, CS:2 * CS], w, xh[k][1][:, CS:2 * CS],
                                 start=(k == 0), stop=(k == KB - 1))
            if db == 4:
                ot = evict_store(db, ps01, 0, store=False)
                evict_store(db, ps23, 1, ot=ot, store=False)
                nc.scalar.dma_start(outT[db * 128:(db + 1) * 128, :], ot[:])
            else:
                # split the last store across both queues for a short tail
                ot = evict_store(db, ps01, 0, eng=nc.sync)
                evict_store(db, ps23, 1, ot=ot, eng=nc.scalar)

    nc.compile()
    return nc


_PROGRAM_CACHE = {}


def _get_program(mode=None):
    if mode is None:
        mode = MODE
    if mode not in _PROGRAM_CACHE:
        _PROGRAM_CACHE[mode] = build_program(mode)
    return _PROGRAM_CACHE[mode]


def make_in_maps(text_features, acoustic_features, Wt, bt, Wa, ba, Wv, bv):
    """Host-side sharding + layout prep: per-batch X^T k-slabs, k-major Wt
    slices, and the host-fused bias terms (Wav = Wa @ Wv,
    bias0 = bt + bv + ba @ Wv), in the exact SBUF tile layouts."""
    bf16 = ml_dtypes.bfloat16
    text_features = np.asarray(text_features, dtype=np.float32)
    # xt[b, p, k*S + s] = X[b, s, k*128+p]
    xt_all = (text_features
              .reshape(B, S, KB, 128)
              .transpose(0, 3, 2, 1)
              .astype(bf16)
              .reshape(B, 128, KB * S))

    wt_packed = (np.asarray(Wt, dtype=np.float32)
                 .reshape(KB, 128, D)
                 .transpose(1, 0, 2)
                 .astype(bf16)
                 .reshape(128, KB * D))

    Wa = np.asarray(Wa, dtype=np.float32)
    Wv = np.asarray(Wv, dtype=np.float32)
    wav = Wa @ Wv                                   # [16, D]
    wavT2 = np.ascontiguousarray(
        wav.reshape(16, DB, 128).transpose(2, 1, 0).reshape(128, DB * 16))
    bias0 = (np.asarray(bt, dtype=np.float32)
             + np.asarray(bv, dtype=np.float32)
             + np.asarray(ba, dtype=np.float32) @ Wv)
    b0t = np.ascontiguousarray(bias0.reshape(DB, 128).T)

    shared = {
        "wt": np.ascontiguousarray(wt_packed),
        "wavT2": wavT2,
        "b0t": b0t,
    }
    acoustic_features = np.asarray(acoustic_features, dtype=np.float32)
    in_maps = []
    for b in range(N_CORES):
        m = dict(shared)
        m["xt"] = np.ascontiguousarray(xt_all[b])
        m["acB"] = np.ascontiguousarray(
            np.broadcast_to(acoustic_features[b], (128, 16)))
        in_maps.append(m)
    return in_maps


def kernel(text_features, acoustic_features, Wt, bt, Wa, ba, Wq, bq, Wk, bk,
           Wv, bv, **_unused):
    nc = _get_program()
    in_maps = make_in_maps(text_features, acoustic_features, Wt, bt, Wa, ba,
                           Wv, bv)
    res = run_bass_kernel_spmd(nc, in_maps, list(range(N_CORES))).results
    out = np.empty((B, S, D), dtype=np.float32)
    for b in range(N_CORES):
        out[b] = np.asarray(res[b]["outT"], dtype=np.float32).T
    return out
